# revision 21
# baseline (speedup 1.0000x reference)
"""Multi-head attention (RoPE on k/v) Bass kernel for 8 TRN2 NeuronCores.

Sharding: tensor-parallel over heads (2 heads/core, both batches) for the
QKV projections + attention; one AllToAll redistributes context to a
row-sharded output projection. All matmuls in fp16 (fp32 PSUM accum).

v2: per-kt software pipeline. PSUM budget (8 banks): scores [128,2,512]
x2 bufs (4), ctx accumulators cA/cB (2), proj (1), shared swap/transpose/
outproj bank (1). One 1024-elem exp per kt keeps ACT streaming; batch-1
projections / v-transposes / output-projection chunks are emitted as
fine-grained fillers inside the attention loop so the PE stream stays
gap-free (p-state ramp).

Self-contained: hardcodes shapes from the problem spec.
"""
import os
import sys
import types

import numpy as np


def _install_ntff_hook():
    """antenv.axon_hooks is missing from this image; synthesize it so
    run_bass_kernel_spmd(trace=True) works. Harmless when trace=False."""
    if "antenv.axon_hooks" in sys.modules:
        return
    try:
        from trn_agent_boot.trn_boot import _ntff_profile_via_ctypes

        hook = _ntff_profile_via_ctypes("/opt/axon/libaxon_pjrt.so")
    except Exception:
        hook = None
    mod = types.ModuleType("antenv.axon_hooks")
    mod._hook = hook
    mod.get_axon_ntff_profile_hook = lambda: mod._hook
    mod.set_axon_ntff_profile_hook = lambda h: setattr(mod, "_hook", h)
    sys.modules["antenv.axon_hooks"] = mod
    try:
        import antenv

        antenv.axon_hooks = mod
    except Exception:
        pass


_install_ntff_hook()

import concourse.bass as bass  # noqa: E402
import concourse.mybir as mybir  # noqa: E402
import concourse.tile as tile  # noqa: E402
from concourse import bacc  # noqa: E402
from concourse.bass import ds  # noqa: E402
from concourse.bass_utils import run_bass_kernel_spmd  # noqa: E402

B, S, D, H = 2, 2048, 1024, 16
R = B * S              # 4096 flattened rows
NC = 8                 # cores
HPC = H // NC          # 2 heads per core
CW = D // NC           # 128 ctx cols per core
DH = D // H            # 64 head dim
RW = R // NC           # 512 output rows per core
F32 = mybir.dt.float32
F32R = mybir.dt.float16  # matmul dtype (fp16 = full PE rate)
FP = np.float32

KT = D // 128   # 8 contraction tiles for projections
RT = R // 512   # 8 row tiles
RTB = RT // B   # 4 row tiles per batch
QT = S // 512   # 4 q tiles per batch
ST = S // 128   # 16 k tiles per batch
ERING = 4       # exp-output ring depth (kt slots)


def _perm_local() -> np.ndarray:
    """Within a core's 128-col slice: [A-evens, A-odds, B-evens, B-odds]."""
    a_ev = np.arange(0, 64, 2)
    a_od = np.arange(1, 64, 2)
    return np.concatenate([a_ev, a_od, 64 + a_ev, 64 + a_od])


def _build_program():
    nc = bacc.Bacc("TRN2", target_bir_lowering=False, debug=False, num_devices=NC)

    # ---- external I/O ----
    xq_d = nc.dram_tensor("xq", [D, R], F32R, kind="ExternalInput").ap()
    xk_d = nc.dram_tensor("xk", [D, R], F32R, kind="ExternalInput").ap()
    xv_d = nc.dram_tensor("xv", [D, R], F32R, kind="ExternalInput").ap()
    wq_d = nc.dram_tensor("wq", [D, CW], F32R, kind="ExternalInput").ap()
    wk_d = nc.dram_tensor("wk", [D, CW], F32R, kind="ExternalInput").ap()
    wv_d = nc.dram_tensor("wv", [D, CW], F32R, kind="ExternalInput").ap()
    wp_d = nc.dram_tensor("wp", [D, D], F32R, kind="ExternalInput").ap()
    cs_d = nc.dram_tensor("cs", [128, R], F32R, kind="ExternalInput").ap()
    sn_d = nc.dram_tensor("sn", [128, R], F32R, kind="ExternalInput").ap()
    sw_d = nc.dram_tensor("sw", [128, 128], F32R, kind="ExternalInput").ap()
    id_d = nc.dram_tensor("ident", [128, 128], F32R, kind="ExternalInput").ap()
    on_d = nc.dram_tensor("ones", [128, 64], F32R, kind="ExternalInput").ap()
    bi_d = nc.dram_tensor("bias", [128, D], F32, kind="ExternalInput").ap()
    out_d = nc.dram_tensor("out", [RW, D], F32, kind="ExternalOutput").ap()

    with tile.TileContext(nc) as tc:
        with (
            tc.tile_pool(name="const", bufs=1) as const,
            tc.tile_pool(name="persist", bufs=1) as persist,
            tc.tile_pool(name="dram", bufs=1, space="DRAM") as dram,
            tc.tile_pool(name="wqkv", bufs=1) as wpool,
            tc.tile_pool(name="trig", bufs=1) as trig,
            tc.tile_pool(name="xin", bufs=5) as xin,
            tc.tile_pool(name="vtmp", bufs=1) as vtmp,
            tc.tile_pool(name="rope", bufs=3) as rp,
            tc.tile_pool(name="epool", bufs=1) as ep,
            tc.tile_pool(name="norm", bufs=2) as npl,
            tc.tile_pool(name="oio", bufs=2) as oio,
            tc.tile_pool(name="pp", bufs=1, space="PSUM") as pp,
            tc.tile_pool(name="scp", bufs=1, space="PSUM") as scp,
            tc.tile_pool(name="cpsum", bufs=1, space="PSUM") as cp,
        ):
            # ---- constants / weights (scalar DMA queue: idle during head,
            # keeps the sync queue free for bulk x loads) ----
            sw_sb = const.tile([128, 128], F32R)
            nc.scalar.dma_start(sw_sb[:], sw_d[:])
            id_sb = const.tile([128, 128], F32R)
            nc.scalar.dma_start(id_sb[:], id_d[:])
            on_sb = const.tile([128, 64], F32R)
            nc.scalar.dma_start(on_sb[:], on_d[:])
            wk_sb = wpool.tile([128, KT, CW], F32R)
            nc.scalar.dma_start(wk_sb[:], wk_d.rearrange("(kt p) m -> p kt m", p=128))
            wv_sb = wpool.tile([128, KT, CW], F32R)
            nc.scalar.dma_start(wv_sb[:], wv_d.rearrange("(kt p) m -> p kt m", p=128))
            wq_sb = wpool.tile([128, KT, CW], F32R)
            nc.scalar.dma_start(wq_sb[:], wq_d.rearrange("(kt p) m -> p kt m", p=128))
            cs_sb = trig.tile([128, R], F32R)
            sn_sb = trig.tile([128, R], F32R)
            bi_sb = const.tile([128, D], F32)
            wp_sb = const.tile([128, KT, D], F32R)

            qpT = persist.tile([128, R], F32R)
            kpT = persist.tile([128, R], F32R)
            vaug = persist.tile([128, R // 128, 256], F32R)
            nc.vector.tensor_copy(
                vaug[:, :, 64:128],
                on_sb[:, :, None].rearrange("p o n -> p n o").to_broadcast((128, R // 128, 64)),
            )
            nc.vector.tensor_copy(
                vaug[:, :, 192:256],
                on_sb[:, :, None].rearrange("p o n -> p n o").to_broadcast((128, R // 128, 64)),
            )
            vpT = vtmp.tile([128, R], F32R)

            # exp-output ring: [128, ERING kt slots, 2 heads, 512]
            eab = ep.tile([128, ERING, 2, 512], F32R)

            _chunk_rows = [128, 128, 128, 64, 64]
            a2a_ins = [dram.tile([NC, 128, _chunk_rows[p]], F32R,
                                 name=f"a2ain{p}", tag=f"a2ain{p}")
                       for p in range(5)]
            a2a_outs = [dram.tile([NC, 128, _chunk_rows[p]], F32R,
                                  name=f"a2aout{p}", tag=f"a2aout{p}")
                        for p in range(5)]
            CHUNKS = [(0, 2), (2, 4), (4, 6), (6, 7), (7, 8)]  # shard groups

            # ---------------- x prefetch machinery ----------------
            # x loaded in quarter-batch tiles [128, KT, 1024] (2KB packets) on
            # the gpsimd DMA queue so small latency-critical DMAs (a2a staging,
            # zlo, lh) flow freely on the sync queue.
            # quarter = (ti, qh): rows [qh*1024, (qh+1)*1024) of tensor ti.
            QORD = [(0, 0), (1, 0), (2, 0), (0, 1), (1, 1), (2, 1),
                    (0, 2), (0, 3), (2, 2), (1, 2), (1, 3), (2, 3)]
            # consumption order of (ti, rt) proj tiles; ti: 0=k 1=v 2=q
            order = ([(0, 0), (0, 1), (1, 0), (2, 0)]          # head
                     + [(0, 2), (0, 3), (1, 1), (2, 1),        # b0-attn fill
                        (1, 2), (2, 2), (1, 3), (2, 3)]
                     + [(0, rt) for rt in range(RTB, RT)]      # k b1
                     + [(2, 4), (2, 5)]                        # q b1 rt4-5
                     + [(1, rt) for rt in range(RTB, RT)]      # v b1
                     + [(2, 6), (2, 7)])                       # q b1 rt6-7
            prefetched = {}
            qstate = {"next": 0, "done": 0}
            quarter_pending = {}   # (ti, qh) -> tiles not yet popped

            def request_quarters():
                while (qstate["next"] < len(QORD)
                       and qstate["next"] < qstate["done"] + 4):
                    qi = qstate["next"]
                    ti, qh = QORD[qi]
                    xd = (xk_d, xv_d, xq_d)[ti]
                    x_sb = xin.tile([128, KT, 1024], F32R, tag="xin", name="x_sb")
                    # b0 quarters (head) on sync; b1 quarters on the scalar
                    # queue so attention-phase staging DMAs on sync never
                    # queue behind bulk x traffic.
                    eng = nc.sync if qi < 6 else nc.scalar
                    eng.dma_start(
                        x_sb[:],
                        xd.rearrange("(kt p) r -> p kt r", p=128)
                          [:, :, ds(qh * 1024, 1024)],
                    )
                    prefetched[(ti, qh)] = x_sb
                    quarter_pending[(ti, qh)] = 2
                    qstate["next"] += 1

            def pop_x(ti, rt):
                qh = rt // 2
                x_sb = prefetched[(ti, qh)]
                quarter_pending[(ti, qh)] -= 1
                if quarter_pending[(ti, qh)] == 0:
                    del prefetched[(ti, qh)]
                    qstate["done"] += 1
                return x_sb[:, :, ds((rt % 2) * 512, 512)]

            def proj_gen(pos):
                """Generator of PE-sized steps for projection tile order[pos]."""
                ti, rt = order[pos]
                request_quarters()
                x_sb = pop_x(ti, rt)
                wsb = (wk_sb, wv_sb, wq_sb)[ti]
                dest = (kpT, vpT, qpT)[ti]
                rsl = ds(rt * 512, 512)
                ps = pp.tile([128, 512], F32, tag="proj", bufs=1, name="ps")
                for g in range(4):
                    for j in range(2):
                        kt = 2 * g + j
                        nc.tensor.matmul(
                            ps[:], wsb[:, kt], x_sb[:, kt],
                            start=(kt == 0), stop=(kt == KT - 1),
                        )
                    yield
                if ti == 2:  # q: plain copy
                    nc.vector.tensor_copy(dest[:, rsl], ps[:])
                    yield
                    return
                # k/v: RoPE
                raw = rp.tile([128, 512], F32R, tag="raw", name="raw")
                nc.vector.tensor_copy(raw[:], ps[:])
                sps = pp.tile([128, 512], F32, tag="po", bufs=1, name="sps")
                nc.tensor.matmul(sps[:], sw_sb[:], raw[:], start=True, stop=True)
                yield
                t1 = rp.tile([128, 512], F32R, tag="t1", name="t1")
                nc.gpsimd.tensor_tensor(
                    t1[:], raw[:], cs_sb[:, rsl], mybir.AluOpType.mult)
                t2 = rp.tile([128, 512], F32R, tag="t2", name="t2")
                nc.vector.tensor_tensor(
                    t2[:], sps[:], sn_sb[:, rsl], mybir.AluOpType.mult)
                yield
                nc.gpsimd.tensor_tensor(
                    dest[:, rsl], t1[:], t2[:], mybir.AluOpType.add)
                yield

            def vtrans_gen(cstart, cstop):
                """Transpose vpT into vaug for ct in [cstart, cstop), 4/step."""
                for c0 in range(cstart, cstop, 4):
                    tp4 = pp.tile([128, 4, 128], F32R, tag="po", bufs=1, name="tp4")
                    for i in range(4):
                        nc.tensor.transpose(
                            tp4[:, i], vpT[:, ds((c0 + i) * 128, 128)], id_sb[:])
                    nc.vector.tensor_copy(vaug[:, c0:c0 + 4, 0:64], tp4[:, :, 0:64])
                    nc.vector.tensor_copy(
                        vaug[:, c0:c0 + 4, 128:192], tp4[:, :, 64:128])
                    yield

            def outproj_gen(ck):
                """Output projection for a2a chunk ck."""
                a, b = CHUNKS[ck]
                nrows = (b - a) * 64
                lh = oio.tile([128, NC, 128], F32R, tag="lh", bufs=1, name="lh")
                nc.sync.dma_start(
                    lh[:, :, 0:nrows],
                    a2a_outs[ck][:].rearrange("j p r -> p j r"),
                )
                yield
                for oc in range(2):
                    po = pp.tile([128, 512], F32, tag="po", bufs=1, name="po")
                    for j in range(NC):
                        nc.tensor.matmul(
                            po[0:nrows], lh[:, j, 0:nrows],
                            wp_sb[:, j, ds(oc * 512, 512)],
                            start=(j == 0), stop=(j == NC - 1))
                        if j % 4 == 3:
                            yield
                    ob = oio.tile([128, 512], F32, tag="ob", name="ob")
                    nc.vector.tensor_tensor(
                        ob[0:nrows], po[0:nrows], bi_sb[0:nrows, ds(oc * 512, 512)],
                        mybir.AluOpType.add)
                    nc.sync.dma_start(
                        out_d[ds(a * 64, nrows), ds(oc * 512, 512)], ob[0:nrows])
                    yield

            # filler queue: generators consumed between attention matmuls
            fillers = []

            def consume(n):
                taken = 0
                while taken < n and fillers:
                    try:
                        next(fillers[0])
                        taken += 1
                    except StopIteration:
                        fillers.pop(0)

            def drain_all():
                while fillers:
                    consume(1)

            # ---------------- attention ----------------
            def emit_attn_qt(bb, qt, fill_per_kt):
                qsl = ds(bb * S + qt * 512, 512)
                cA = cp.tile([128, 512], F32, tag="cA", bufs=1, name="cA")
                cB = cp.tile([128, 512], F32, tag="cB", bufs=1, name="cB")

                def ctx_kt(kt):
                    ct = bb * ST + kt
                    e = eab[:, kt % ERING]
                    nc.tensor.matmul(cA[:], vaug[:, ct, 0:128], e[:, 0],
                                     start=(kt == 0), stop=(kt == ST - 1))
                    nc.tensor.matmul(cB[:], vaug[:, ct, 128:256], e[:, 1],
                                     start=(kt == 0), stop=(kt == ST - 1))

                # ctx lags scores/exp by 2 kt so its exp dependency is always
                # satisfied by the time the PE reaches it (no per-kt stall).
                for kt in range(ST):
                    ksl = ds(bb * S + kt * 128, 128)
                    sc = scp.tile([128, 2, 512], F32, tag="sc", bufs=2, name="sc")
                    nc.tensor.matmul(sc[:, 0], kpT[0:64, ksl], qpT[0:64, qsl],
                                     start=True, stop=True)
                    nc.tensor.matmul(sc[:, 1], kpT[64:128, ksl], qpT[64:128, qsl],
                                     start=True, stop=True)
                    nc.scalar.activation(
                        eab[:, kt % ERING], sc[:],
                        mybir.ActivationFunctionType.Exp, scale=0.125)
                    if kt >= 2:
                        ctx_kt(kt - 2)
                    consume(fill_per_kt)
                consume(2)
                ctx_kt(ST - 2)
                ctx_kt(ST - 1)

                # ---- normalize + stage for AllToAll ----
                sAs = npl.tile([128, 512], F32, tag="sAs", name="sAs")
                nc.vector.tensor_copy(sAs[:], cA[:])
                sBs = npl.tile([128, 512], F32, tag="sBs", name="sBs")
                nc.vector.tensor_copy(sBs[:], cB[:])
                zlo = npl.tile([64, 1024], F32, tag="zlo", bufs=1, name="zlo")
                nc.sync.dma_start(zlo[:, 0:512], sAs[64:128, :])
                nc.sync.dma_start(zlo[:, 512:1024], sBs[64:128, :])
                zr = npl.tile([64, 1024], F32, tag="zr", bufs=1, name="zr")
                nc.vector.reciprocal_approx_fast(zr[:], zlo[:])
                ctxA = npl.tile([64, 512], F32R, tag="ctxA", name="ctxA")
                nc.vector.tensor_tensor(
                    ctxA[:], sAs[0:64], zr[:, 0:512], mybir.AluOpType.mult)
                ctxB = npl.tile([64, 512], F32R, tag="ctxB", name="ctxB")
                nc.vector.tensor_tensor(
                    ctxB[:], sBs[0:64], zr[:, 512:1024], mybir.AluOpType.mult)
                shard = bb * QT + qt
                ck = next(i for i, (a, b) in enumerate(CHUNKS) if a <= shard < b)
                a, b = CHUNKS[ck]
                rsl2 = ds((shard - a) * 64, 64)
                nc.sync.dma_start(
                    a2a_ins[ck][:, 0:64, rsl2].rearrange("j p r -> p j r"),
                    ctxA[:].rearrange("p (j r) -> p j r", j=NC))
                nc.sync.dma_start(
                    a2a_ins[ck][:, 64:128, rsl2].rearrange("j p r -> p j r"),
                    ctxB[:].rearrange("p (j r) -> p j r", j=NC))
                if shard == b - 1:
                    nc.gpsimd.collective_compute(
                        "AllToAll",
                        mybir.AluOpType.bypass,
                        replica_groups=[list(range(NC))],
                        ins=[a2a_ins[ck].opt()],
                        outs=[a2a_outs[ck].opt()],
                    )

            # ---------------- main schedule ----------------
            # head: k rt0-1, v rt0, q rt0, vtrans ct0-3 — emitted direct
            request_quarters()
            nc.scalar.dma_start(cs_sb[:], cs_d[:])
            nc.scalar.dma_start(sn_sb[:], sn_d[:])
            for pos in range(4):
                for _ in proj_gen(pos):
                    pass
            nc.scalar.dma_start(bi_sb[:], bi_d[:])
            nc.scalar.dma_start(wp_sb[:], wp_d.rearrange("(kt p) o -> p kt o", p=128))
            for _ in vtrans_gen(0, 4):
                pass

            # b0 attention fillers:
            # order idx: 4-11 k2,k3,v1,q1,v2,q2,v3,q3 | 12-15 k4-7 |
            #            16-17 q4-5 | 18-21 v4-7 | 22-23 q6-7
            fillers.append(proj_gen(4))                       # k2
            fillers.append(proj_gen(5))                       # k3
            fillers.append(proj_gen(6))                       # v1
            fillers.append(vtrans_gen(4, 8))
            fillers.append(proj_gen(7))                       # q1
            fillers.append(proj_gen(8))                       # v2
            fillers.append(vtrans_gen(8, 12))
            fillers.append(proj_gen(9))                       # q2
            fillers.append(proj_gen(10))                      # v3
            fillers.append(vtrans_gen(12, 16))
            fillers.append(proj_gen(11))                      # q3
            for qt in range(QT):
                if qt == 1:
                    fillers.extend(proj_gen(pos) for pos in range(12, 16))
                if qt == 2:
                    fillers.extend(proj_gen(pos) for pos in range(16, 22))
                    fillers.append(vtrans_gen(ST, ST + 8))
                if qt == 3:
                    fillers.append(vtrans_gen(ST + 8, 2 * ST))
                    fillers.extend(proj_gen(pos) for pos in range(22, 24))
                emit_attn_qt(0, qt, 3 if qt == 0 else 2)
            drain_all()

            # b1 attention; fillers: outproj chunks as a2a results land
            for qt in range(QT):
                if qt == 0:
                    fillers.append(outproj_gen(0))
                if qt == 1:
                    fillers.append(outproj_gen(1))
                if qt == 3:
                    fillers.append(outproj_gen(2))
                emit_attn_qt(1, qt, 2)
                if qt == 3:
                    fillers.append(outproj_gen(3))
            drain_all()
            for _ in outproj_gen(4):
                pass

    nc.compile()
    return nc


_PROGRAM = None


def _get_program():
    global _PROGRAM
    if _PROGRAM is None:
        _PROGRAM = _build_program()
    return _PROGRAM


def _host_prep(q, k, v, Wq, Wk, Wv, Wp, bp):
    """Build the 8 per-core input maps."""
    rr = lambda a: np.ascontiguousarray(a, dtype=np.float32).astype(np.float16)
    xqT = rr(q.reshape(R, D).T)
    xkT = rr(k.reshape(R, D).T)
    xvT = rr(v.reshape(R, D).T)

    pl = _perm_local()
    perm_global = np.concatenate([128 * c + pl for c in range(NC)])
    wpT = rr(np.ascontiguousarray(Wp.T[perm_global, :]))

    # trig tables
    half = D // 2
    pos = np.arange(S, dtype=np.float64)
    theta = 1.0 / (10000.0 ** (2.0 * np.arange(half, dtype=np.float64) / D))
    ang = pos[:, None] * theta[None, :]          # [S, half]
    cosf = np.cos(ang).astype(FP)                # [S, half]
    sinf = np.sin(ang).astype(FP)

    sw = np.zeros((128, 128), np.float16)
    for m in range(128):
        p = (m + 32) % 64 + 64 * (m // 64)
        sw[p, m] = 1.0
    ident = np.eye(128, dtype=np.float16)
    ones = np.ones((128, 64), np.float16)
    bias = np.broadcast_to(bp.astype(FP), (128, D)).copy()

    in_maps = []
    for c in range(NC):
        cols = 128 * c + pl
        wq_c = rr(np.ascontiguousarray(Wq[cols, :].T))
        wk_c = rr(np.ascontiguousarray(Wk[cols, :].T))
        wv_c = rr(np.ascontiguousarray(Wv[cols, :].T))
        # pair index per partition p (see _perm_local ordering)
        j = np.empty(128, np.int64)
        j[0:32] = 64 * c + np.arange(32)
        j[32:64] = 64 * c + np.arange(32)
        j[64:96] = 64 * c + 32 + np.arange(32)
        j[96:128] = 64 * c + 32 + np.arange(32)
        cs1 = cosf[:, j].T                        # [128, S]
        sn1 = sinf[:, j].T.copy()
        sn1[0:32] *= -1.0
        sn1[64:96] *= -1.0
        cs = np.tile(cs1, (1, B)).astype(np.float16)      # [128, R]
        sn = np.tile(sn1, (1, B)).astype(np.float16)
        in_maps.append({
            "xq": xqT, "xk": xkT, "xv": xvT,
            "wq": wq_c, "wk": wk_c, "wv": wv_c,
            "wp": wpT, "cs": cs, "sn": sn,
            "sw": sw, "ident": ident, "ones": ones, "bias": bias,
        })
    return in_maps


def run(inputs, trace=False, trace_cores=None):
    nc = _get_program()
    in_maps = _host_prep(**inputs)
    res = run_bass_kernel_spmd(
        nc, in_maps, core_ids=list(range(NC)), trace=trace,
        trace_cores=trace_cores,
    )
    outs = np.stack([res.results[c]["out"] for c in range(NC)])  # [c, 512, D]
    # local row (128p + 64g' + i) on core c == global row 512*(2p+g') + 64c + i
    lo = outs.reshape(NC, NC, 64, D)              # [core, (2p,g'), i, D]
    full = lo.transpose(1, 0, 2, 3).reshape(B, S, D).astype(np.float32)
    return full, res


def kernel(**inputs) -> np.ndarray:
    trace = bool(int(os.environ.get("TRN_TRACE", "0")))
    full, res = run(inputs, trace=trace)
    if trace and res.exec_time_ns is not None:
        print(f"HW exec time: {res.exec_time_ns} ns")
    return full


# revision 24
# speedup vs baseline: 1.3221x; 1.3221x over previous
"""Multi-head attention (RoPE on k/v) Bass kernel for 8 TRN2 NeuronCores.

Sharding: tensor-parallel over heads (2 heads/core, both batches) for the
QKV projections + attention; one AllToAll redistributes context to a
row-sharded output projection. All matmuls in fp16 (fp32 PSUM accum).

v2: per-kt software pipeline. PSUM budget (8 banks): scores [128,2,512]
x2 bufs (4), ctx accumulators cA/cB (2), proj (1), shared swap/transpose/
outproj bank (1). One 1024-elem exp per kt keeps ACT streaming; batch-1
projections / v-transposes / output-projection chunks are emitted as
fine-grained fillers inside the attention loop so the PE stream stays
gap-free (p-state ramp).

Self-contained: hardcodes shapes from the problem spec.
"""
import os
import sys
import types

import numpy as np


def _install_ntff_hook():
    """antenv.axon_hooks is missing from this image; synthesize it so
    run_bass_kernel_spmd(trace=True) works. Harmless when trace=False."""
    if "antenv.axon_hooks" in sys.modules:
        return
    try:
        from trn_agent_boot.trn_boot import _ntff_profile_via_ctypes

        hook = _ntff_profile_via_ctypes("/opt/axon/libaxon_pjrt.so")
    except Exception:
        hook = None
    mod = types.ModuleType("antenv.axon_hooks")
    mod._hook = hook
    mod.get_axon_ntff_profile_hook = lambda: mod._hook
    mod.set_axon_ntff_profile_hook = lambda h: setattr(mod, "_hook", h)
    sys.modules["antenv.axon_hooks"] = mod
    try:
        import antenv

        antenv.axon_hooks = mod
    except Exception:
        pass


_install_ntff_hook()

import concourse.bass as bass  # noqa: E402
import concourse.mybir as mybir  # noqa: E402
import concourse.tile as tile  # noqa: E402
from concourse import bacc  # noqa: E402
from concourse.bass import ds  # noqa: E402
from concourse.bass_utils import run_bass_kernel_spmd  # noqa: E402

B, S, D, H = 2, 2048, 1024, 16
R = B * S              # 4096 flattened rows
NC = 8                 # cores
HPC = H // NC          # 2 heads per core
CW = D // NC           # 128 ctx cols per core
DH = D // H            # 64 head dim
RW = R // NC           # 512 output rows per core
F32 = mybir.dt.float32
F32R = mybir.dt.float16  # matmul dtype (fp16 = full PE rate)
FP = np.float32

KT = D // 128   # 8 contraction tiles for projections
RT = R // 512   # 8 row tiles
RTB = RT // B   # 4 row tiles per batch
QT = S // 512   # 4 q tiles per batch
ST = S // 128   # 16 k tiles per batch
ERING = 4       # exp-output ring depth (kt slots)


def _perm_local() -> np.ndarray:
    """Within a core's 128-col slice: [A-evens, A-odds, B-evens, B-odds]."""
    a_ev = np.arange(0, 64, 2)
    a_od = np.arange(1, 64, 2)
    return np.concatenate([a_ev, a_od, 64 + a_ev, 64 + a_od])


def _build_program():
    nc = bacc.Bacc("TRN2", target_bir_lowering=False, debug=False, num_devices=NC)

    # ---- external I/O ----
    xq_d = nc.dram_tensor("xq", [D, R], F32R, kind="ExternalInput").ap()
    xk_d = nc.dram_tensor("xk", [D, R], F32R, kind="ExternalInput").ap()
    xv_d = nc.dram_tensor("xv", [D, R], F32R, kind="ExternalInput").ap()
    wq_d = nc.dram_tensor("wq", [D, CW], F32R, kind="ExternalInput").ap()
    wk_d = nc.dram_tensor("wk", [D, CW], F32R, kind="ExternalInput").ap()
    wv_d = nc.dram_tensor("wv", [D, CW], F32R, kind="ExternalInput").ap()
    wp_d = nc.dram_tensor("wp", [D, D], F32R, kind="ExternalInput").ap()
    cs_d = nc.dram_tensor("cs", [128, R], F32R, kind="ExternalInput").ap()
    sn_d = nc.dram_tensor("sn", [128, R], F32R, kind="ExternalInput").ap()
    sw_d = nc.dram_tensor("sw", [128, 128], F32R, kind="ExternalInput").ap()
    id_d = nc.dram_tensor("ident", [128, 128], F32R, kind="ExternalInput").ap()
    on_d = nc.dram_tensor("ones", [128, 64], F32R, kind="ExternalInput").ap()
    bi_d = nc.dram_tensor("bias", [128, D], F32, kind="ExternalInput").ap()
    out_d = nc.dram_tensor("out", [RW, D], F32, kind="ExternalOutput").ap()

    with tile.TileContext(nc) as tc:
        with (
            tc.tile_pool(name="const", bufs=1) as const,
            tc.tile_pool(name="persist", bufs=1) as persist,
            tc.tile_pool(name="dram", bufs=1, space="DRAM") as dram,
            tc.tile_pool(name="wqkv", bufs=1) as wpool,
            tc.tile_pool(name="trig", bufs=1) as trig,
            tc.tile_pool(name="xin", bufs=5) as xin,
            tc.tile_pool(name="vtmp", bufs=1) as vtmp,
            tc.tile_pool(name="rope", bufs=3) as rp,
            tc.tile_pool(name="epool", bufs=1) as ep,
            tc.tile_pool(name="norm", bufs=2) as npl,
            tc.tile_pool(name="oio", bufs=2) as oio,
            tc.tile_pool(name="pp", bufs=1, space="PSUM") as pp,
            tc.tile_pool(name="scp", bufs=1, space="PSUM") as scp,
            tc.tile_pool(name="cpsum", bufs=1, space="PSUM") as cp,
        ):
            # ---- constants / weights (scalar DMA queue: idle during head,
            # keeps the sync queue free for bulk x loads) ----
            sw_sb = const.tile([128, 128], F32R)
            nc.scalar.dma_start(sw_sb[:], sw_d[:])
            id_sb = const.tile([128, 128], F32R)
            nc.scalar.dma_start(id_sb[:], id_d[:])
            on_sb = const.tile([128, 64], F32R)
            nc.scalar.dma_start(on_sb[:], on_d[:])
            wk_sb = wpool.tile([128, KT, CW], F32R)
            nc.scalar.dma_start(wk_sb[:], wk_d.rearrange("(kt p) m -> p kt m", p=128))
            wv_sb = wpool.tile([128, KT, CW], F32R)
            nc.scalar.dma_start(wv_sb[:], wv_d.rearrange("(kt p) m -> p kt m", p=128))
            wq_sb = wpool.tile([128, KT, CW], F32R)
            nc.scalar.dma_start(wq_sb[:], wq_d.rearrange("(kt p) m -> p kt m", p=128))
            cs_sb = trig.tile([128, R], F32R)
            sn_sb = trig.tile([128, R], F32R)
            bi_sb = const.tile([128, D], F32)
            wp_sb = const.tile([128, KT, D], F32R)

            qpT = persist.tile([128, R], F32R)
            kpT = persist.tile([128, R], F32R)
            vaug = persist.tile([128, R // 128, 256], F32R)
            nc.vector.tensor_copy(
                vaug[:, :, 64:128],
                on_sb[:, :, None].rearrange("p o n -> p n o").to_broadcast((128, R // 128, 64)),
            )
            nc.vector.tensor_copy(
                vaug[:, :, 192:256],
                on_sb[:, :, None].rearrange("p o n -> p n o").to_broadcast((128, R // 128, 64)),
            )
            vpT = vtmp.tile([128, R], F32R)

            # exp-output ring: [128, ERING kt slots, 2 heads, 512]
            eab = ep.tile([128, ERING, 2, 512], F32R)

            _chunk_rows = [128, 128, 128, 64, 64]
            a2a_ins = [dram.tile([NC, 128, _chunk_rows[p]], F32R,
                                 name=f"a2ain{p}", tag=f"a2ain{p}")
                       for p in range(5)]
            a2a_outs = [dram.tile([NC, 128, _chunk_rows[p]], F32R,
                                  name=f"a2aout{p}", tag=f"a2aout{p}")
                        for p in range(5)]
            CHUNKS = [(0, 2), (2, 4), (4, 6), (6, 7), (7, 8)]  # shard groups

            # ---------------- x prefetch machinery ----------------
            # x loaded in quarter-batch tiles [128, KT, 1024] (2KB packets) on
            # the gpsimd DMA queue so small latency-critical DMAs (a2a staging,
            # zlo, lh) flow freely on the sync queue.
            # quarter = (ti, qh): rows [qh*1024, (qh+1)*1024) of tensor ti.
            QORD = [(0, 0), (1, 0), (2, 0), (0, 1), (1, 1), (2, 1),
                    (0, 2), (0, 3), (2, 2), (1, 2), (1, 3), (2, 3)]
            # consumption order of (ti, rt) proj tiles; ti: 0=k 1=v 2=q.
            # All b0 k/v tiles are in the head: scores/ctx read kpT and vaug
            # as PE *stationary* operands, and concurrent stationary writes
            # during qt0 produced NaN (moving-operand tiles like qpT are safe
            # to fill concurrently).
            order = ([(0, 0), (0, 1), (1, 0), (2, 0),          # head
                      (0, 2), (0, 3), (1, 1), (1, 2), (1, 3)]
                     + [(2, 1), (2, 2), (2, 3)]                # b0-attn fill
                     + [(0, rt) for rt in range(RTB, RT)]      # k b1
                     + [(2, 4), (2, 5)]                        # q b1 rt4-5
                     + [(1, rt) for rt in range(RTB, RT)]      # v b1
                     + [(2, 6), (2, 7)])                       # q b1 rt6-7
            prefetched = {}
            qstate = {"next": 0, "done": 0}
            quarter_pending = {}   # (ti, qh) -> tiles not yet popped

            def request_quarters():
                while (qstate["next"] < len(QORD)
                       and qstate["next"] < qstate["done"] + 4):
                    qi = qstate["next"]
                    ti, qh = QORD[qi]
                    xd = (xk_d, xv_d, xq_d)[ti]
                    x_sb = xin.tile([128, KT, 1024], F32R, tag="xin", name="x_sb")
                    # All on sync (hw-dge). The request window (4) + xin bufs
                    # (5) guarantee a dma_start never waits for a slot, so the
                    # sync queue never head-of-line blocks.
                    nc.sync.dma_start(
                        x_sb[:],
                        xd.rearrange("(kt p) r -> p kt r", p=128)
                          [:, :, ds(qh * 1024, 1024)],
                    )
                    prefetched[(ti, qh)] = x_sb
                    quarter_pending[(ti, qh)] = 2
                    qstate["next"] += 1

            def pop_x(ti, rt):
                qh = rt // 2
                x_sb = prefetched[(ti, qh)]
                quarter_pending[(ti, qh)] -= 1
                if quarter_pending[(ti, qh)] == 0:
                    del prefetched[(ti, qh)]
                    qstate["done"] += 1
                return x_sb[:, :, ds((rt % 2) * 512, 512)]

            def proj_gen(pos):
                """Generator of PE-sized steps for projection tile order[pos]."""
                ti, rt = order[pos]
                request_quarters()
                x_sb = pop_x(ti, rt)
                wsb = (wk_sb, wv_sb, wq_sb)[ti]
                dest = (kpT, vpT, qpT)[ti]
                rsl = ds(rt * 512, 512)
                ps = pp.tile([128, 512], F32, tag="proj", bufs=1, name="ps")
                for g in range(4):
                    for j in range(2):
                        kt = 2 * g + j
                        nc.tensor.matmul(
                            ps[:], wsb[:, kt], x_sb[:, kt],
                            start=(kt == 0), stop=(kt == KT - 1),
                        )
                    yield
                if ti == 2:  # q: plain copy
                    nc.vector.tensor_copy(dest[:, rsl], ps[:])
                    yield
                    return
                # k/v: RoPE
                raw = rp.tile([128, 512], F32R, tag="raw", name="raw")
                nc.vector.tensor_copy(raw[:], ps[:])
                sps = pp.tile([128, 512], F32, tag="po", bufs=1, name="sps")
                nc.tensor.matmul(sps[:], sw_sb[:], raw[:], start=True, stop=True)
                yield
                t1 = rp.tile([128, 512], F32R, tag="t1", name="t1")
                nc.gpsimd.tensor_tensor(
                    t1[:], raw[:], cs_sb[:, rsl], mybir.AluOpType.mult)
                t2 = rp.tile([128, 512], F32R, tag="t2", name="t2")
                nc.vector.tensor_tensor(
                    t2[:], sps[:], sn_sb[:, rsl], mybir.AluOpType.mult)
                yield
                nc.gpsimd.tensor_tensor(
                    dest[:, rsl], t1[:], t2[:], mybir.AluOpType.add)
                yield

            def vtrans_gen(cstart, cstop):
                """Transpose vpT into vaug for ct in [cstart, cstop), 4/step."""
                for c0 in range(cstart, cstop, 4):
                    tp4 = pp.tile([128, 4, 128], F32R, tag="po", bufs=1, name="tp4")
                    for i in range(4):
                        nc.tensor.transpose(
                            tp4[:, i], vpT[:, ds((c0 + i) * 128, 128)], id_sb[:])
                    nc.vector.tensor_copy(vaug[:, c0:c0 + 4, 0:64], tp4[:, :, 0:64])
                    nc.vector.tensor_copy(
                        vaug[:, c0:c0 + 4, 128:192], tp4[:, :, 64:128])
                    yield

            def outproj_gen(ck):
                """Output projection for a2a chunk ck."""
                a, b = CHUNKS[ck]
                nrows = (b - a) * 64
                lh = oio.tile([128, NC, 128], F32R, tag="lh", bufs=1, name="lh")
                nc.sync.dma_start(
                    lh[:, :, 0:nrows],
                    a2a_outs[ck][:].rearrange("j p r -> p j r"),
                )
                yield
                for oc in range(2):
                    po = pp.tile([128, 512], F32, tag="po", bufs=1, name="po")
                    for j in range(NC):
                        nc.tensor.matmul(
                            po[0:nrows], lh[:, j, 0:nrows],
                            wp_sb[:, j, ds(oc * 512, 512)],
                            start=(j == 0), stop=(j == NC - 1))
                        if j % 4 == 3:
                            yield
                    ob = oio.tile([128, 512], F32, tag="ob", name="ob")
                    nc.vector.tensor_tensor(
                        ob[0:nrows], po[0:nrows], bi_sb[0:nrows, ds(oc * 512, 512)],
                        mybir.AluOpType.add)
                    nc.sync.dma_start(
                        out_d[ds(a * 64, nrows), ds(oc * 512, 512)], ob[0:nrows])
                    yield

            # filler queue: generators consumed between attention matmuls
            fillers = []

            def consume(n):
                taken = 0
                while taken < n and fillers:
                    try:
                        next(fillers[0])
                        taken += 1
                    except StopIteration:
                        fillers.pop(0)

            def drain_all():
                while fillers:
                    consume(1)

            # ---------------- attention ----------------
            def emit_attn_qt(bb, qt, fill_per_kt):
                qsl = ds(bb * S + qt * 512, 512)
                cA = cp.tile([128, 512], F32, tag="cA", bufs=1, name="cA")
                cB = cp.tile([128, 512], F32, tag="cB", bufs=1, name="cB")

                def ctx_kt(kt):
                    ct = bb * ST + kt
                    e = eab[:, kt % ERING]
                    nc.tensor.matmul(cA[:], vaug[:, ct, 0:128], e[:, 0],
                                     start=(kt == 0), stop=(kt == ST - 1))
                    nc.tensor.matmul(cB[:], vaug[:, ct, 128:256], e[:, 1],
                                     start=(kt == 0), stop=(kt == ST - 1))

                # ctx lags scores/exp by 2 kt so its exp dependency is always
                # satisfied by the time the PE reaches it (no per-kt stall).
                for kt in range(ST):
                    ksl = ds(bb * S + kt * 128, 128)
                    sc = scp.tile([128, 2, 512], F32, tag="sc", bufs=2, name="sc")
                    nc.tensor.matmul(sc[:, 0], kpT[0:64, ksl], qpT[0:64, qsl],
                                     start=True, stop=True)
                    nc.tensor.matmul(sc[:, 1], kpT[64:128, ksl], qpT[64:128, qsl],
                                     start=True, stop=True)
                    nc.scalar.activation(
                        eab[:, kt % ERING], sc[:],
                        mybir.ActivationFunctionType.Exp, scale=0.125)
                    if kt >= 2:
                        ctx_kt(kt - 2)
                    consume(fill_per_kt)
                consume(2)
                ctx_kt(ST - 2)
                ctx_kt(ST - 1)

                # ---- normalize + stage for AllToAll ----
                sAs = npl.tile([128, 512], F32, tag="sAs", name="sAs")
                nc.vector.tensor_copy(sAs[:], cA[:])
                sBs = npl.tile([128, 512], F32, tag="sBs", name="sBs")
                nc.vector.tensor_copy(sBs[:], cB[:])
                zlo = npl.tile([64, 1024], F32, tag="zlo", bufs=1, name="zlo")
                nc.sync.dma_start(zlo[:, 0:512], sAs[64:128, :])
                nc.sync.dma_start(zlo[:, 512:1024], sBs[64:128, :])
                zr = npl.tile([64, 1024], F32, tag="zr", bufs=1, name="zr")
                nc.vector.reciprocal_approx_fast(zr[:], zlo[:])
                ctxA = npl.tile([64, 512], F32R, tag="ctxA", name="ctxA")
                nc.vector.tensor_tensor(
                    ctxA[:], sAs[0:64], zr[:, 0:512], mybir.AluOpType.mult)
                ctxB = npl.tile([64, 512], F32R, tag="ctxB", name="ctxB")
                nc.vector.tensor_tensor(
                    ctxB[:], sBs[0:64], zr[:, 512:1024], mybir.AluOpType.mult)
                shard = bb * QT + qt
                ck = next(i for i, (a, b) in enumerate(CHUNKS) if a <= shard < b)
                a, b = CHUNKS[ck]
                rsl2 = ds((shard - a) * 64, 64)
                nc.sync.dma_start(
                    a2a_ins[ck][:, 0:64, rsl2].rearrange("j p r -> p j r"),
                    ctxA[:].rearrange("p (j r) -> p j r", j=NC))
                nc.sync.dma_start(
                    a2a_ins[ck][:, 64:128, rsl2].rearrange("j p r -> p j r"),
                    ctxB[:].rearrange("p (j r) -> p j r", j=NC))
                if shard == b - 1:
                    nc.gpsimd.collective_compute(
                        "AllToAll",
                        mybir.AluOpType.bypass,
                        replica_groups=[list(range(NC))],
                        ins=[a2a_ins[ck].opt()],
                        outs=[a2a_outs[ck].opt()],
                    )

            # ---------------- main schedule ----------------
            # head: k rt0-1, v rt0, q rt0, vtrans ct0-3 — emitted direct
            request_quarters()
            nc.scalar.dma_start(cs_sb[:], cs_d[:])
            nc.scalar.dma_start(sn_sb[:], sn_d[:])
            for pos in range(9):
                for _ in proj_gen(pos):
                    pass
            nc.scalar.dma_start(bi_sb[:], bi_d[:])
            nc.scalar.dma_start(wp_sb[:], wp_d.rearrange("(kt p) o -> p kt o", p=128))
            for _ in vtrans_gen(0, 16):
                pass

            # b0 attention fillers:
            # order idx: 9-11 q1-3 | 12-15 k4-7 | 16-17 q4-5 |
            #            18-21 v4-7 | 22-23 q6-7
            fillers.extend(proj_gen(pos) for pos in range(9, 12))   # q1-3
            for qt in range(QT):
                if qt == 1:
                    fillers.extend(proj_gen(pos) for pos in range(12, 16))
                if qt == 2:
                    fillers.extend(proj_gen(pos) for pos in range(16, 22))
                    fillers.append(vtrans_gen(ST, ST + 8))
                if qt == 3:
                    fillers.append(vtrans_gen(ST + 8, 2 * ST))
                    fillers.extend(proj_gen(pos) for pos in range(22, 24))
                emit_attn_qt(0, qt, 3 if qt == 0 else 2)
            drain_all()

            # b1 attention; fillers: outproj chunks as a2a results land
            for qt in range(QT):
                if qt == 0:
                    fillers.append(outproj_gen(0))
                if qt == 1:
                    fillers.append(outproj_gen(1))
                if qt == 3:
                    fillers.append(outproj_gen(2))
                emit_attn_qt(1, qt, 2)
                if qt == 3:
                    fillers.append(outproj_gen(3))
            drain_all()
            for _ in outproj_gen(4):
                pass

    nc.compile()
    return nc


_PROGRAM = None


def _get_program():
    global _PROGRAM
    if _PROGRAM is None:
        _PROGRAM = _build_program()
    return _PROGRAM


def _host_prep(q, k, v, Wq, Wk, Wv, Wp, bp):
    """Build the 8 per-core input maps."""
    rr = lambda a: np.ascontiguousarray(a, dtype=np.float32).astype(np.float16)
    xqT = rr(q.reshape(R, D).T)
    xkT = rr(k.reshape(R, D).T)
    xvT = rr(v.reshape(R, D).T)

    pl = _perm_local()
    perm_global = np.concatenate([128 * c + pl for c in range(NC)])
    wpT = rr(np.ascontiguousarray(Wp.T[perm_global, :]))

    # trig tables
    half = D // 2
    pos = np.arange(S, dtype=np.float64)
    theta = 1.0 / (10000.0 ** (2.0 * np.arange(half, dtype=np.float64) / D))
    ang = pos[:, None] * theta[None, :]          # [S, half]
    cosf = np.cos(ang).astype(FP)                # [S, half]
    sinf = np.sin(ang).astype(FP)

    sw = np.zeros((128, 128), np.float16)
    for m in range(128):
        p = (m + 32) % 64 + 64 * (m // 64)
        sw[p, m] = 1.0
    ident = np.eye(128, dtype=np.float16)
    ones = np.ones((128, 64), np.float16)
    bias = np.broadcast_to(bp.astype(FP), (128, D)).copy()

    in_maps = []
    for c in range(NC):
        cols = 128 * c + pl
        wq_c = rr(np.ascontiguousarray(Wq[cols, :].T))
        wk_c = rr(np.ascontiguousarray(Wk[cols, :].T))
        wv_c = rr(np.ascontiguousarray(Wv[cols, :].T))
        # pair index per partition p (see _perm_local ordering)
        j = np.empty(128, np.int64)
        j[0:32] = 64 * c + np.arange(32)
        j[32:64] = 64 * c + np.arange(32)
        j[64:96] = 64 * c + 32 + np.arange(32)
        j[96:128] = 64 * c + 32 + np.arange(32)
        cs1 = cosf[:, j].T                        # [128, S]
        sn1 = sinf[:, j].T.copy()
        sn1[0:32] *= -1.0
        sn1[64:96] *= -1.0
        cs = np.tile(cs1, (1, B)).astype(np.float16)      # [128, R]
        sn = np.tile(sn1, (1, B)).astype(np.float16)
        in_maps.append({
            "xq": xqT, "xk": xkT, "xv": xvT,
            "wq": wq_c, "wk": wk_c, "wv": wv_c,
            "wp": wpT, "cs": cs, "sn": sn,
            "sw": sw, "ident": ident, "ones": ones, "bias": bias,
        })
    return in_maps


def run(inputs, trace=False, trace_cores=None):
    nc = _get_program()
    in_maps = _host_prep(**inputs)
    res = run_bass_kernel_spmd(
        nc, in_maps, core_ids=list(range(NC)), trace=trace,
        trace_cores=trace_cores,
    )
    outs = np.stack([res.results[c]["out"] for c in range(NC)])  # [c, 512, D]
    # local row (128p + 64g' + i) on core c == global row 512*(2p+g') + 64c + i
    lo = outs.reshape(NC, NC, 64, D)              # [core, (2p,g'), i, D]
    full = lo.transpose(1, 0, 2, 3).reshape(B, S, D).astype(np.float32)
    return full, res


def kernel(**inputs) -> np.ndarray:
    trace = bool(int(os.environ.get("TRN_TRACE", "0")))
    full, res = run(inputs, trace=trace)
    if trace and res.exec_time_ns is not None:
        print(f"HW exec time: {res.exec_time_ns} ns")
    return full


# revision 32
# speedup vs baseline: 1.3255x; 1.0025x over previous
"""Multi-head attention (RoPE on k/v) Bass kernel for 8 TRN2 NeuronCores.

Sharding: tensor-parallel over heads (2 heads/core, both batches) for the
QKV projections + attention; one AllToAll redistributes context to a
row-sharded output projection. All matmuls in fp16 (fp32 PSUM accum).

v2: per-kt software pipeline. PSUM budget (8 banks): scores [128,2,512]
x2 bufs (4), ctx accumulators cA/cB (2), proj (1), shared swap/transpose/
outproj bank (1). One 1024-elem exp per kt keeps ACT streaming; batch-1
projections / v-transposes / output-projection chunks are emitted as
fine-grained fillers inside the attention loop so the PE stream stays
gap-free (p-state ramp).

Self-contained: hardcodes shapes from the problem spec.
"""
import os
import sys
import types

import numpy as np


def _install_ntff_hook():
    """antenv.axon_hooks is missing from this image; synthesize it so
    run_bass_kernel_spmd(trace=True) works. Harmless when trace=False."""
    if "antenv.axon_hooks" in sys.modules:
        return
    try:
        from trn_agent_boot.trn_boot import _ntff_profile_via_ctypes

        hook = _ntff_profile_via_ctypes("/opt/axon/libaxon_pjrt.so")
    except Exception:
        hook = None
    mod = types.ModuleType("antenv.axon_hooks")
    mod._hook = hook
    mod.get_axon_ntff_profile_hook = lambda: mod._hook
    mod.set_axon_ntff_profile_hook = lambda h: setattr(mod, "_hook", h)
    sys.modules["antenv.axon_hooks"] = mod
    try:
        import antenv

        antenv.axon_hooks = mod
    except Exception:
        pass


_install_ntff_hook()

import concourse.bass as bass  # noqa: E402
import concourse.mybir as mybir  # noqa: E402
import concourse.tile as tile  # noqa: E402
from concourse import bacc  # noqa: E402
from concourse.bass import ds  # noqa: E402
from concourse.bass_utils import run_bass_kernel_spmd  # noqa: E402

B, S, D, H = 2, 2048, 1024, 16
R = B * S              # 4096 flattened rows
NC = 8                 # cores
HPC = H // NC          # 2 heads per core
CW = D // NC           # 128 ctx cols per core
DH = D // H            # 64 head dim
RW = R // NC           # 512 output rows per core
F32 = mybir.dt.float32
F32R = mybir.dt.float16  # matmul dtype (fp16 = full PE rate)
FP = np.float32

KT = D // 128   # 8 contraction tiles for projections
RT = R // 512   # 8 row tiles
RTB = RT // B   # 4 row tiles per batch
QT = S // 512   # 4 q tiles per batch
ST = S // 128   # 16 k tiles per batch
ERING = 4       # exp-output ring depth (kt slots)


def _perm_local() -> np.ndarray:
    """Within a core's 128-col slice: [A-evens, A-odds, B-evens, B-odds]."""
    a_ev = np.arange(0, 64, 2)
    a_od = np.arange(1, 64, 2)
    return np.concatenate([a_ev, a_od, 64 + a_ev, 64 + a_od])


def _build_program():
    nc = bacc.Bacc("TRN2", target_bir_lowering=False, debug=False, num_devices=NC)

    # ---- external I/O ----
    xq_d = nc.dram_tensor("xq", [D, R], F32R, kind="ExternalInput").ap()
    xk_d = nc.dram_tensor("xk", [D, R], F32R, kind="ExternalInput").ap()
    xv_d = nc.dram_tensor("xv", [D, R], F32R, kind="ExternalInput").ap()
    wq_d = nc.dram_tensor("wq", [D, CW], F32R, kind="ExternalInput").ap()
    wk_d = nc.dram_tensor("wk", [D, CW], F32R, kind="ExternalInput").ap()
    wv_d = nc.dram_tensor("wv", [D, CW], F32R, kind="ExternalInput").ap()
    F8 = mybir.dt.float8e4
    wp_d = nc.dram_tensor("wp", [64, KT * 2 * D], F8, kind="ExternalInput").ap()
    cs_d = nc.dram_tensor("cs", [128, R], F32R, kind="ExternalInput").ap()
    sn_d = nc.dram_tensor("sn", [128, R], F32R, kind="ExternalInput").ap()
    sw_d = nc.dram_tensor("sw", [128, 128], F32R, kind="ExternalInput").ap()
    id_d = nc.dram_tensor("ident", [128, 128], F32R, kind="ExternalInput").ap()
    on_d = nc.dram_tensor("ones", [128, 64], F32R, kind="ExternalInput").ap()
    bi_d = nc.dram_tensor("bias", [128, D], F32, kind="ExternalInput").ap()
    out_d = nc.dram_tensor("out", [RW, D], F32, kind="ExternalOutput").ap()

    with tile.TileContext(nc) as tc:
        with (
            tc.tile_pool(name="const", bufs=1) as const,
            tc.tile_pool(name="persist", bufs=1) as persist,
            tc.tile_pool(name="dram", bufs=1, space="DRAM") as dram,
            tc.tile_pool(name="wqkv", bufs=1) as wpool,
            tc.tile_pool(name="trig", bufs=1) as trig,
            tc.tile_pool(name="xin", bufs=5) as xin,
            tc.tile_pool(name="vtmp", bufs=1) as vtmp,
            tc.tile_pool(name="rope", bufs=3) as rp,
            tc.tile_pool(name="epool", bufs=1) as ep,
            tc.tile_pool(name="norm", bufs=2) as npl,
            tc.tile_pool(name="oio", bufs=2) as oio,
            tc.tile_pool(name="pp", bufs=1, space="PSUM") as pp,
            tc.tile_pool(name="scp", bufs=1, space="PSUM") as scp,
            tc.tile_pool(name="cpsum", bufs=1, space="PSUM") as cp,
        ):
            # ---- constants / weights (scalar DMA queue: idle during head,
            # keeps the sync queue free for bulk x loads) ----
            sw_sb = const.tile([128, 128], F32R)
            nc.scalar.dma_start(sw_sb[:], sw_d[:])
            id_sb = const.tile([128, 128], F32R)
            nc.scalar.dma_start(id_sb[:], id_d[:])
            on_sb = const.tile([128, 64], F32R)
            nc.scalar.dma_start(on_sb[:], on_d[:])
            wk_sb = wpool.tile([128, KT, CW], F32R)
            nc.scalar.dma_start(wk_sb[:], wk_d.rearrange("(kt p) m -> p kt m", p=128))
            wv_sb = wpool.tile([128, KT, CW], F32R)
            nc.scalar.dma_start(wv_sb[:], wv_d.rearrange("(kt p) m -> p kt m", p=128))
            wq_sb = wpool.tile([128, KT, CW], F32R)
            nc.scalar.dma_start(wq_sb[:], wq_d.rearrange("(kt p) m -> p kt m", p=128))
            cs_sb = trig.tile([128, R], F32R)
            sn_sb = trig.tile([128, R], F32R)
            bi_sb = const.tile([128, D], F32)
            # DoubleRow fp8 layout: [pair-partition, kt-block, pair-idx, out]
            wp_sb = const.tile([64, KT, 2, D], F8)

            qpT = persist.tile([128, R], F32R)
            kpT = persist.tile([128, R], F32R)
            vaug = persist.tile([128, R // 128, 256], F32R)
            nc.vector.tensor_copy(
                vaug[:, :, 64:128],
                on_sb[:, :, None].rearrange("p o n -> p n o").to_broadcast((128, R // 128, 64)),
            )
            nc.vector.tensor_copy(
                vaug[:, :, 192:256],
                on_sb[:, :, None].rearrange("p o n -> p n o").to_broadcast((128, R // 128, 64)),
            )
            vpT = vtmp.tile([128, R], F32R)

            # exp-output ring: [128, ERING kt slots, 2 heads, 512]
            eab = ep.tile([128, ERING, 2, 512], F32R)

            _chunk_rows = [128, 128, 128, 64, 64]
            a2a_ins = [dram.tile([NC, 128, _chunk_rows[p]], F8,
                                 name=f"a2ain{p}", tag=f"a2ain{p}")
                       for p in range(5)]
            a2a_outs = [dram.tile([NC, 128, _chunk_rows[p]], F8,
                                  name=f"a2aout{p}", tag=f"a2aout{p}")
                        for p in range(5)]
            CHUNKS = [(0, 2), (2, 4), (4, 6), (6, 7), (7, 8)]  # shard groups

            # ---------------- x prefetch machinery ----------------
            # x loaded in quarter-batch tiles [128, KT, 1024] (2KB packets) on
            # the gpsimd DMA queue so small latency-critical DMAs (a2a staging,
            # zlo, lh) flow freely on the sync queue.
            # quarter = (ti, qh): rows [qh*1024, (qh+1)*1024) of tensor ti.
            QORD = [(0, 0), (1, 0), (2, 0), (0, 1), (1, 1), (2, 1),
                    (0, 2), (0, 3), (2, 2), (1, 2), (1, 3), (2, 3)]
            # consumption order of (ti, rt) proj tiles; ti: 0=k 1=v 2=q.
            # All b0 k/v tiles are in the head: scores/ctx read kpT and vaug
            # as PE *stationary* operands, and concurrent stationary writes
            # during qt0 produced NaN (moving-operand tiles like qpT are safe
            # to fill concurrently).
            order = ([(0, 0), (0, 1), (1, 0), (2, 0),          # head
                      (0, 2), (0, 3), (1, 1), (1, 2), (1, 3)]
                     + [(2, 1), (2, 2), (2, 3)]                # b0-attn fill
                     + [(0, rt) for rt in range(RTB, RT)]      # k b1
                     + [(2, 4), (2, 5)]                        # q b1 rt4-5
                     + [(1, rt) for rt in range(RTB, RT)]      # v b1
                     + [(2, 6), (2, 7)])                       # q b1 rt6-7
            prefetched = {}
            qstate = {"next": 0, "done": 0}
            quarter_pending = {}   # (ti, qh) -> tiles not yet popped

            def request_quarters():
                while (qstate["next"] < len(QORD)
                       and qstate["next"] < qstate["done"] + 4):
                    qi = qstate["next"]
                    ti, qh = QORD[qi]
                    xd = (xk_d, xv_d, xq_d)[ti]
                    x_sb = xin.tile([128, KT, 1024], F32R, tag="xin", name="x_sb")
                    # All on sync (hw-dge). The request window (4) + xin bufs
                    # (5) guarantee a dma_start never waits for a slot, so the
                    # sync queue never head-of-line blocks.
                    nc.sync.dma_start(
                        x_sb[:],
                        xd.rearrange("(kt p) r -> p kt r", p=128)
                          [:, :, ds(qh * 1024, 1024)],
                    )
                    prefetched[(ti, qh)] = x_sb
                    quarter_pending[(ti, qh)] = 2
                    qstate["next"] += 1

            def pop_x(ti, rt):
                qh = rt // 2
                x_sb = prefetched[(ti, qh)]
                quarter_pending[(ti, qh)] -= 1
                if quarter_pending[(ti, qh)] == 0:
                    del prefetched[(ti, qh)]
                    qstate["done"] += 1
                return x_sb[:, :, ds((rt % 2) * 512, 512)]

            def proj_gen(pos):
                """Generator of PE-sized steps for projection tile order[pos]."""
                ti, rt = order[pos]
                request_quarters()
                x_sb = pop_x(ti, rt)
                wsb = (wk_sb, wv_sb, wq_sb)[ti]
                dest = (kpT, vpT, qpT)[ti]
                rsl = ds(rt * 512, 512)
                ps = pp.tile([128, 512], F32, tag="proj", bufs=1, name="ps")
                for g in range(4):
                    for j in range(2):
                        kt = 2 * g + j
                        nc.tensor.matmul(
                            ps[:], wsb[:, kt], x_sb[:, kt],
                            start=(kt == 0), stop=(kt == KT - 1),
                        )
                    yield
                if ti == 2:  # q: plain copy
                    nc.vector.tensor_copy(dest[:, rsl], ps[:])
                    yield
                    return
                # k/v: RoPE
                raw = rp.tile([128, 512], F32R, tag="raw", name="raw")
                nc.vector.tensor_copy(raw[:], ps[:])
                sps = pp.tile([128, 512], F32, tag="po", bufs=1, name="sps")
                nc.tensor.matmul(sps[:], sw_sb[:], raw[:], start=True, stop=True)
                yield
                t1 = rp.tile([128, 512], F32R, tag="t1", name="t1")
                nc.gpsimd.tensor_tensor(
                    t1[:], raw[:], cs_sb[:, rsl], mybir.AluOpType.mult)
                t2 = rp.tile([128, 512], F32R, tag="t2", name="t2")
                nc.vector.tensor_tensor(
                    t2[:], sps[:], sn_sb[:, rsl], mybir.AluOpType.mult)
                yield
                nc.gpsimd.tensor_tensor(
                    dest[:, rsl], t1[:], t2[:], mybir.AluOpType.add)
                yield

            def vtrans_gen(cstart, cstop):
                """Transpose vpT into vaug for ct in [cstart, cstop), 4/step."""
                for c0 in range(cstart, cstop, 4):
                    tp4 = pp.tile([128, 4, 128], F32R, tag="po", bufs=1, name="tp4")
                    for i in range(4):
                        nc.tensor.transpose(
                            tp4[:, i], vpT[:, ds((c0 + i) * 128, 128)], id_sb[:])
                    nc.vector.tensor_copy(vaug[:, c0:c0 + 4, 0:64], tp4[:, :, 0:64])
                    nc.vector.tensor_copy(
                        vaug[:, c0:c0 + 4, 128:192], tp4[:, :, 64:128])
                    yield

            def outproj_gen(ck):
                """Output projection for a2a chunk ck."""
                a, b = CHUNKS[ck]
                nrows = (b - a) * 64
                lh = oio.tile([64, NC, 2, 128], F8, tag="lh", bufs=1, name="lh")
                nc.sync.dma_start(
                    lh[:, :, :, 0:nrows],
                    a2a_outs[ck][:].rearrange("j (i p) r -> p j i r", i=2),
                )
                yield
                for oc in range(2):
                    po = pp.tile([128, 512], F32, tag="po", bufs=1, name="po")
                    for j in range(NC):
                        nc.tensor.matmul(
                            po[0:nrows], lh[:, j, :, 0:nrows],
                            wp_sb[:, j, :, ds(oc * 512, 512)],
                            start=(j == 0), stop=(j == NC - 1),
                            perf_mode=mybir.MatmulPerfMode.DoubleRow)
                        if j % 4 == 3:
                            yield
                    ob = oio.tile([128, 512], F32, tag="ob", name="ob")
                    nc.vector.tensor_tensor(
                        ob[0:nrows], po[0:nrows], bi_sb[0:nrows, ds(oc * 512, 512)],
                        mybir.AluOpType.add)
                    nc.sync.dma_start(
                        out_d[ds(a * 64, nrows), ds(oc * 512, 512)], ob[0:nrows])
                    yield

            # filler queue: generators consumed between attention matmuls
            fillers = []

            def consume(n):
                taken = 0
                while taken < n and fillers:
                    try:
                        next(fillers[0])
                        taken += 1
                    except StopIteration:
                        fillers.pop(0)

            def drain_all():
                while fillers:
                    consume(1)

            # ---------------- attention ----------------
            def emit_attn_qt(bb, qt, fill_per_kt):
                qsl = ds(bb * S + qt * 512, 512)
                cA = cp.tile([128, 512], F32, tag="cA", bufs=1, name="cA")
                cB = cp.tile([128, 512], F32, tag="cB", bufs=1, name="cB")

                def ctx_kt(kt):
                    ct = bb * ST + kt
                    e = eab[:, kt % ERING]
                    nc.tensor.matmul(cA[:], vaug[:, ct, 0:128], e[:, 0],
                                     start=(kt == 0), stop=(kt == ST - 1))
                    nc.tensor.matmul(cB[:], vaug[:, ct, 128:256], e[:, 1],
                                     start=(kt == 0), stop=(kt == ST - 1))

                # ctx lags scores/exp by 2 kt so its exp dependency is always
                # satisfied by the time the PE reaches it (no per-kt stall).
                for kt in range(ST):
                    ksl = ds(bb * S + kt * 128, 128)
                    sc = scp.tile([128, 2, 512], F32, tag="sc", bufs=2, name="sc")
                    nc.tensor.matmul(sc[:, 0], kpT[0:64, ksl], qpT[0:64, qsl],
                                     start=True, stop=True)
                    nc.tensor.matmul(sc[:, 1], kpT[64:128, ksl], qpT[64:128, qsl],
                                     start=True, stop=True)
                    nc.scalar.activation(
                        eab[:, kt % ERING], sc[:],
                        mybir.ActivationFunctionType.Exp, scale=0.125)
                    if kt >= 2:
                        ctx_kt(kt - 2)
                    consume(fill_per_kt)
                consume(2)
                ctx_kt(ST - 2)
                ctx_kt(ST - 1)

                # ---- normalize + stage for AllToAll ----
                sAs = npl.tile([128, 512], F32, tag="sAs", name="sAs")
                nc.vector.tensor_copy(sAs[:], cA[:])
                sBs = npl.tile([128, 512], F32, tag="sBs", name="sBs")
                nc.vector.tensor_copy(sBs[:], cB[:])
                zlo = npl.tile([64, 1024], F32, tag="zlo", bufs=1, name="zlo")
                nc.sync.dma_start(zlo[:, 0:512], sAs[64:128, :])
                nc.sync.dma_start(zlo[:, 512:1024], sBs[64:128, :])
                zr = npl.tile([64, 1024], F32, tag="zr", bufs=1, name="zr")
                nc.vector.reciprocal_approx_fast(zr[:], zlo[:])
                ctxA = npl.tile([64, 512], F8, tag="ctxA", name="ctxA")
                nc.vector.tensor_tensor(
                    ctxA[:], sAs[0:64], zr[:, 0:512], mybir.AluOpType.mult)
                ctxB = npl.tile([64, 512], F8, tag="ctxB", name="ctxB")
                nc.vector.tensor_tensor(
                    ctxB[:], sBs[0:64], zr[:, 512:1024], mybir.AluOpType.mult)
                shard = bb * QT + qt
                ck = next(i for i, (a, b) in enumerate(CHUNKS) if a <= shard < b)
                a, b = CHUNKS[ck]
                rsl2 = ds((shard - a) * 64, 64)
                nc.sync.dma_start(
                    a2a_ins[ck][:, 0:64, rsl2].rearrange("j p r -> p j r"),
                    ctxA[:].rearrange("p (j r) -> p j r", j=NC))
                nc.sync.dma_start(
                    a2a_ins[ck][:, 64:128, rsl2].rearrange("j p r -> p j r"),
                    ctxB[:].rearrange("p (j r) -> p j r", j=NC))
                if shard == b - 1:
                    nc.gpsimd.collective_compute(
                        "AllToAll",
                        mybir.AluOpType.bypass,
                        replica_groups=[list(range(NC))],
                        ins=[a2a_ins[ck].opt()],
                        outs=[a2a_outs[ck].opt()],
                    )

            # ---------------- main schedule ----------------
            # head: k rt0-1, v rt0, q rt0, vtrans ct0-3 — emitted direct
            request_quarters()
            nc.scalar.dma_start(cs_sb[:], cs_d[:])
            nc.scalar.dma_start(sn_sb[:], sn_d[:])
            for pos in range(9):
                for _ in proj_gen(pos):
                    pass
            nc.scalar.dma_start(bi_sb[:], bi_d[:])
            nc.scalar.dma_start(
                wp_sb[:], wp_d.rearrange("p (kt i n) -> p kt i n", kt=KT, i=2))
            for _ in vtrans_gen(0, 16):
                pass

            # b0 attention fillers:
            # order idx: 9-11 q1-3 | 12-15 k4-7 | 16-17 q4-5 |
            #            18-21 v4-7 | 22-23 q6-7
            fillers.extend(proj_gen(pos) for pos in range(9, 12))   # q1-3
            for qt in range(QT):
                if qt == 1:
                    fillers.extend(proj_gen(pos) for pos in range(12, 16))
                if qt == 2:
                    fillers.extend(proj_gen(pos) for pos in range(16, 22))
                    fillers.append(vtrans_gen(ST, ST + 8))
                if qt == 3:
                    fillers.append(vtrans_gen(ST + 8, 2 * ST))
                    fillers.extend(proj_gen(pos) for pos in range(22, 24))
                emit_attn_qt(0, qt, 3 if qt == 0 else 2)
            drain_all()

            # b1 attention; fillers: outproj chunks as a2a results land
            for qt in range(QT):
                if qt == 0:
                    fillers.append(outproj_gen(0))
                if qt == 1:
                    fillers.append(outproj_gen(1))
                if qt == 3:
                    fillers.append(outproj_gen(2))
                emit_attn_qt(1, qt, 2)
                if qt == 3:
                    fillers.append(outproj_gen(3))
            drain_all()
            for _ in outproj_gen(4):
                pass

    nc.compile()
    return nc


_PROGRAM = None


def _get_program():
    global _PROGRAM
    if _PROGRAM is None:
        _PROGRAM = _build_program()
    return _PROGRAM


def _host_prep(q, k, v, Wq, Wk, Wv, Wp, bp):
    """Build the 8 per-core input maps."""
    rr = lambda a: np.ascontiguousarray(a, dtype=np.float32).astype(np.float16)
    xqT = rr(q.reshape(R, D).T)
    xkT = rr(k.reshape(R, D).T)
    xvT = rr(v.reshape(R, D).T)

    pl = _perm_local()
    perm_global = np.concatenate([128 * c + pl for c in range(NC)])
    import ml_dtypes
    wpT = np.ascontiguousarray(Wp.T[perm_global, :], dtype=np.float32)
    # DoubleRow fp8 layout: [p, kt*2*D] with row d = kt*128 + i*64 + p
    wp8 = np.ascontiguousarray(
        wpT.reshape(KT, 2, 64, D).transpose(2, 0, 1, 3).reshape(64, KT * 2 * D)
    ).astype(ml_dtypes.float8_e4m3fn)

    # trig tables
    half = D // 2
    pos = np.arange(S, dtype=np.float64)
    theta = 1.0 / (10000.0 ** (2.0 * np.arange(half, dtype=np.float64) / D))
    ang = pos[:, None] * theta[None, :]          # [S, half]
    cosf = np.cos(ang).astype(FP)                # [S, half]
    sinf = np.sin(ang).astype(FP)

    sw = np.zeros((128, 128), np.float16)
    for m in range(128):
        p = (m + 32) % 64 + 64 * (m // 64)
        sw[p, m] = 1.0
    ident = np.eye(128, dtype=np.float16)
    ones = np.ones((128, 64), np.float16)
    bias = np.broadcast_to(bp.astype(FP), (128, D)).copy()

    in_maps = []
    for c in range(NC):
        cols = 128 * c + pl
        wq_c = rr(np.ascontiguousarray(Wq[cols, :].T))
        wk_c = rr(np.ascontiguousarray(Wk[cols, :].T))
        wv_c = rr(np.ascontiguousarray(Wv[cols, :].T))
        # pair index per partition p (see _perm_local ordering)
        j = np.empty(128, np.int64)
        j[0:32] = 64 * c + np.arange(32)
        j[32:64] = 64 * c + np.arange(32)
        j[64:96] = 64 * c + 32 + np.arange(32)
        j[96:128] = 64 * c + 32 + np.arange(32)
        cs1 = cosf[:, j].T                        # [128, S]
        sn1 = sinf[:, j].T.copy()
        sn1[0:32] *= -1.0
        sn1[64:96] *= -1.0
        cs = np.tile(cs1, (1, B)).astype(np.float16)      # [128, R]
        sn = np.tile(sn1, (1, B)).astype(np.float16)
        in_maps.append({
            "xq": xqT, "xk": xkT, "xv": xvT,
            "wq": wq_c, "wk": wk_c, "wv": wv_c,
            "wp": wp8, "cs": cs, "sn": sn,
            "sw": sw, "ident": ident, "ones": ones, "bias": bias,
        })
    return in_maps


def run(inputs, trace=False, trace_cores=None):
    nc = _get_program()
    in_maps = _host_prep(**inputs)
    res = run_bass_kernel_spmd(
        nc, in_maps, core_ids=list(range(NC)), trace=trace,
        trace_cores=trace_cores,
    )
    outs = np.stack([res.results[c]["out"] for c in range(NC)])  # [c, 512, D]
    # local row (128p + 64g' + i) on core c == global row 512*(2p+g') + 64c + i
    lo = outs.reshape(NC, NC, 64, D)              # [core, (2p,g'), i, D]
    full = lo.transpose(1, 0, 2, 3).reshape(B, S, D).astype(np.float32)
    return full, res


def kernel(**inputs) -> np.ndarray:
    trace = bool(int(os.environ.get("TRN_TRACE", "0")))
    full, res = run(inputs, trace=trace)
    if trace and res.exec_time_ns is not None:
        print(f"HW exec time: {res.exec_time_ns} ns")
    return full
